# revision 1
# baseline (speedup 1.0000x reference)
"""Trainium2 Bass kernel: segment mean+max pooling (AnchorHeightPart).

reference semantics (per (n, s) row, P=16 parts, k=512 elements, c=128 chans):
  pooled[c, p] = segsum(x*vm)[c,p]/max(segcount(vm)[p],1)
               + where(patchcount[p]>0, max(segmax(x)[c,p], -100), 0)

Device algorithm (per core, data-parallel over n: 4 n-batches/core):
  counting-sort each row's 512 columns by label entirely on-device
  (one-hot -> cumsum scan -> positions -> wrapped inverse via local_scatter),
  permute columns with gpsimd ap_gather, then two segmented
  tensor_tensor_scans (max with -1e30 boundary injection; sum with 0/1
  boundary mask), gather scan values at segment-end positions, combine.
"""

import os
import sys
from contextlib import ExitStack

import numpy as np

_REPO = "/opt/trn_rl_repo"
if _REPO not in sys.path and os.path.isdir(_REPO):
    sys.path.insert(0, _REPO)

N, C, S, K = 32, 128, 30, 512
P = 16
N_CORES = 8
N_PER_CORE = N // N_CORES          # 4
ROWS = N_PER_CORE * S              # 120 rows per core
BLK = 8                            # rows per label-block
NBLK = ROWS // BLK                 # 15
SH = S // 3                        # s-rows per feats sub-tile (10)

_CACHE = {}


def _consts():
    import ml_dtypes
    bf16 = ml_dtypes.bfloat16
    q = np.arange(128)
    g = q // 16       # row-group of partition
    w = q % 16        # within-group lane (part index / wrap residue)

    c = {}
    c["E8"] = (g[None, :] == np.arange(8)[:, None]).astype(np.float32)          # [8,128]
    c["G2"] = (g[:, None] == g[None, :]).astype(np.float32)                     # [128,128]
    c["T16"] = ((g[:, None] == g[None, :]) & (w[:, None] < w[None, :])).astype(np.float32)
    c["R16"] = (w[:, None] == np.arange(16)[None, :]).astype(np.float32)        # [128,16]
    # EEr[r][q, q'] = (q == 16 r + q'%16): broadcast row-r's 16-part stripe to all 128
    for r in range(8):
        c[f"EEr{r}"] = (q[:, None] == 16 * r + w[None, :]).astype(np.float32)
        c[f"EErb{r}"] = c[f"EEr{r}"].astype(bf16)
        c[f"EEB{r}"] = np.broadcast_to((g == r)[:, None], (128, 128)).astype(np.float32)
    c["iotaP"] = w.astype(np.float32)[:, None]                                  # [128,1]
    c["iotaWn"] = (-w.astype(np.float32))[:, None]                              # [128,1]
    c["SIXT"] = np.full((128, 1), 0.0625, np.float32)
    c["JDATA"] = np.broadcast_to(np.arange(K, dtype=np.int16), (128, K)).copy()
    c["NEG16"] = np.full((128, 16), -1e30, bf16)
    c["ONE16"] = np.ones((128, 16), bf16)
    c["PAT2"] = np.broadcast_to(np.array([0.0, float(K)], np.float32), (128, 2)).copy()
    c["ONE1"] = np.ones((128, 1), np.float32)
    c["NEG1"] = np.full((128, 1), -1.0, np.float32)
    # block-level ends-gather / combine helpers
    c["A3"] = (w[:, None] == w[None, :]).astype(np.float32)                 # [128,128]
    c["GR8"] = (g[:, None] == np.arange(8)[None, :]).astype(np.float32)    # [128,8]
    c["ONES128"] = np.ones((128, 128), np.float32)
    c["I128"] = np.eye(128, dtype=np.float32)
    c["PATa"] = np.broadcast_to((1024.0 * (np.arange(8) % 4).astype(np.float32)), (128, 8)).copy()
    c["PATb"] = c["PATa"] + 512.0
    return c


def build_kernel_body(stk, tc, nc, dram):
    from concourse import mybir
    from concourse.tile_rust import add_dep_helper
    dt = mybir.dt
    Alu = mybir.AluOpType
    Act = mybir.ActivationFunctionType
    f32, i16, i32, bf = dt.float32, dt.int16, dt.int32, dt.bfloat16

    feats_d = dram["feats"]     # [N_PER_CORE, C, S, K] f32
    labels_d = dram["labels"]   # [ROWS, K] f32 (host pre-cast)
    out_d = dram["out"]         # [N_PER_CORE, C, S, P] f32

    cpool = stk.enter_context(tc.tile_pool(name="consts", bufs=1))
    keep = stk.enter_context(tc.tile_pool(name="keep", bufs=NBLK))
    lp = stk.enter_context(tc.tile_pool(name="lp", bufs=2))
    pp = stk.enter_context(tc.tile_pool(name="pp", bufs=1, space="PSUM"))
    ppo = stk.enter_context(tc.tile_pool(name="ppo", bufs=1, space="PSUM"))
    fpool = stk.enter_context(tc.tile_pool(name="feats", bufs=2))
    vp = stk.enter_context(tc.tile_pool(name="vp", bufs=5))
    scpool = stk.enter_context(tc.tile_pool(name="scp", bufs=2))
    ivpool = stk.enter_context(tc.tile_pool(name="ivp", bufs=1, space="PSUM"))
    brpool = stk.enter_context(tc.tile_pool(name="brp", bufs=2, space="PSUM"))
    opool = stk.enter_context(tc.tile_pool(name="outacc", bufs=2))

    def ldconst(name, dtype=f32):
        a = dram[name]
        t = cpool.tile(list(a.shape), dtype, tag=name)
        nc.sync.dma_start(out=t[:], in_=a[:])
        return t

    E8 = ldconst("E8")
    G2 = ldconst("G2")
    T16 = ldconst("T16")
    R16 = ldconst("R16")
    iotaP = ldconst("iotaP")
    iotaWn = ldconst("iotaWn")
    SIXT = ldconst("SIXT")
    JDATA = ldconst("JDATA", dtype=i16)
    NEG16 = ldconst("NEG16", dtype=bf)
    ONE16 = ldconst("ONE16", dtype=bf)
    ONE1 = ldconst("ONE1")
    NEG1 = ldconst("NEG1")
    A3 = ldconst("A3")
    GR8 = ldconst("GR8")
    ONES128 = ldconst("ONES128")
    I128 = ldconst("I128")
    PATa = ldconst("PATa")
    PATb = ldconst("PATb")
    EErb = [ldconst(f"EErb{r}", dtype=bf) for r in range(8)]

    dbg = {}
    KDEBUG = bool(os.environ.get("KDEBUG"))
    def dbg_dump(name, tile_ap):
        if KDEBUG and name in dram:
            nc.sync.dma_start(out=dram[name][:], in_=tile_ap)

    # ---------------- phase 1: label pipeline per block ----------------
    blocks = {}
    scatter_insts = []
    epoch_last_gather = [None]

    def label_block(b):
        Lf8 = lp.tile([BLK, K], f32, tag="Lf8")
        nc.sync.dma_start(out=Lf8[:], in_=labels_d[b * BLK:(b + 1) * BLK, :])
        Lrep = pp.tile([128, K], f32, tag="bigL")
        nc.tensor.matmul(Lrep[:], lhsT=E8[:], rhs=Lf8[:], start=True, stop=True)

        # one-hot: O = (Lrep == p(w))  -- in1 unused under bypass
        O = lp.tile([128, K], f32, tag="O")
        nc.vector.scalar_tensor_tensor(
            out=O[:], in0=Lrep[:], scalar=iotaP[:, 0:1],
            in1=iotaP[:, 0:1].to_broadcast([128, K]),
            op0=Alu.is_equal, op1=Alu.bypass)

        # cumulative count along k
        Cc = lp.tile([128, K], f32, tag="Cc")
        nc.vector.tensor_tensor_scan(
            out=Cc[:], data0=O[:], data1=O[:], initial=0.0,
            op0=Alu.add, op1=Alu.bypass)
        counts = Cc[:, K - 1:K]

        mrgall = ppo.tile([128, 304], f32, tag="mrgall")
        offp = mrgall[:, 0:16]
        nc.tensor.matmul(offp[:, 0:1], lhsT=T16[:], rhs=counts, start=True, stop=True)

        om1 = lp.tile([128, 1], f32, tag="om1")
        nc.vector.tensor_scalar(out=om1[:], in0=offp[:, 0:1], scalar1=-1.0,
                                scalar2=None, op0=Alu.add)
        ends0 = lp.tile([128, 1], f32, tag="ends0")
        nc.vector.tensor_tensor(out=ends0[:], in0=om1[:], in1=counts, op=Alu.add)
        endsc = keep.tile([128, 1], f32, tag="endsc")
        nc.vector.tensor_scalar(out=endsc[:], in0=ends0[:], scalar1=0.0,
                                scalar2=None, op0=Alu.max)

        ctc = lp.tile([128, 1], f32, tag="ctc")
        nc.vector.tensor_scalar(out=ctc[:], in0=counts, scalar1=1.0,
                                scalar2=None, op0=Alu.max)
        recip = lp.tile([128, 1], f32, tag="recip")
        nc.vector.reciprocal(out=recip[:], in_=ctc[:])
        indic = lp.tile([128, 1], f32, tag="indic")
        nc.vector.tensor_scalar(out=indic[:], in0=counts, scalar1=0.0,
                                scalar2=None, op0=Alu.is_gt)

        # diag forms: one ONES128 matmul broadcasts recip/indic over (r,p) free dim
        ridiag = lp.tile([128, 256], f32, tag="ridiag")
        nc.vector.tensor_tensor(out=ridiag[:, 0:128],
                                in0=recip[:, 0:1].to_broadcast([128, 128]),
                                in1=I128[:], op=Alu.mult)
        nc.vector.tensor_tensor(out=ridiag[:, 128:256],
                                in0=indic[:, 0:1].to_broadcast([128, 128]),
                                in1=I128[:], op=Alu.mult)
        ribc = keep.tile([128, 256], f32, tag="ribc")
        mrg = mrgall[:, 32:304]
        nc.tensor.matmul(mrg[:, 16:272], lhsT=ONES128[:], rhs=ridiag[:], start=True, stop=True)
        nc.scalar.copy(out=ribc[:], in_=mrg[:, 16:272])

        # ends transposed to [w-partition, r-free] then block gather idx table
        e8d = lp.tile([128, 8], f32, tag="e8d")
        nc.vector.tensor_tensor(out=e8d[:], in0=endsc[:, 0:1].to_broadcast([128, 8]),
                                in1=GR8[:], op=Alu.mult)
        endsT = mrg[:, 0:16]
        nc.tensor.matmul(endsT[:, 0:8], lhsT=A3[:], rhs=e8d[:], start=True, stop=True)
        eidxf = lp.tile([128, 16], f32, tag="eidxf")
        nc.vector.tensor_tensor(out=eidxf[:].rearrange("q (s m) -> q s m", m=2)[:, :, 0],
                                in0=endsT[:, 0:8], in1=PATa[:], op=Alu.add)
        nc.vector.tensor_tensor(out=eidxf[:].rearrange("q (s m) -> q s m", m=2)[:, :, 1],
                                in0=endsT[:, 0:8], in1=PATb[:], op=Alu.add)
        eidx = keep.tile([128, 16], i16, tag="eidx")
        nc.scalar.activation(out=eidx[:], in_=eidxf[:], func=Act.Copy)

        # positions: posm = (Cc + (off-1)) * O   (masked; zero elsewhere)
        posm = lp.tile([128, K], f32, tag="posm")
        nc.vector.scalar_tensor_tensor(
            out=posm[:], in0=Cc[:], scalar=om1[:, 0:1], in1=O[:],
            op0=Alu.add, op1=Alu.mult)
        posr = pp.tile([128, K], f32, tag="bigP")
        nc.tensor.matmul(posr[:], lhsT=G2[:], rhs=posm[:], start=True, stop=True)

        # wrapped-inverse index build (rounding-mode independent):
        # e = (pos - w)/16 is integer iff partition lane w owns sorted slot pos
        ev = lp.tile([128, K], f32, tag="ev")
        nc.vector.scalar_tensor_tensor(
            out=ev[:], in0=posr[:], scalar=iotaWn[:, 0:1],
            in1=SIXT[:, 0:1].to_broadcast([128, K]),
            op0=Alu.add, op1=Alu.mult)
        ei = lp.tile([128, K], i32, tag="ei")
        nc.scalar.activation(out=ei[:], in_=ev[:], func=Act.Copy)
        efp1 = lp.tile([128, K], f32, tag="efp1")
        nc.scalar.activation(out=efp1[:], in_=ei[:], func=Act.Identity, bias=ONE1[:, 0:1])
        # match = (round(ev) == ev) == (efp1 - 1 == ev), fused with the mult
        match = lp.tile([128, K], f32, tag="match")
        nc.vector.scalar_tensor_tensor(
            out=match[:], in0=efp1[:], scalar=-1.0, in1=ev[:],
            op0=Alu.add, op1=Alu.is_equal)
        idxwf = lp.tile([128, K], f32, tag="idxwf")
        nc.vector.tensor_tensor(out=idxwf[:], in0=match[:], in1=efp1[:], op=Alu.mult)
        idx16 = lp.tile([128, K], i16, tag="idx16")
        nc.scalar.activation(out=idx16[:], in_=idxwf[:], func=Act.Identity, bias=NEG1[:, 0:1])

        inv = keep.tile([128, K // 16], i16, tag="inv")
        sc_i1 = nc.gpsimd.local_scatter(
            out_ap=inv[:], data_ap=JDATA[:], idxs_ap=idx16[:],
            channels=128, num_elems=K // 16, num_idxs=K)

        # boundary stripes from offsets
        offd = lp.tile([128, 16], f32, tag="offd")
        nc.vector.tensor_tensor(out=offd[:], in0=offp[:, 0:1].to_broadcast([128, 16]),
                                in1=R16[:], op=Alu.mult)
        offT = mrgall[:, 16:32]
        nc.tensor.matmul(offT[:], lhsT=G2[:], rhs=offd[:], start=True, stop=True)
        offT16 = lp.tile([128, 16], i16, tag="offT16")
        nc.scalar.activation(out=offT16[:], in_=offT[:], func=Act.Copy)

        bneg = keep.tile([128, K], bf, tag="bneg")
        sc_i2 = nc.gpsimd.local_scatter(
            out_ap=bneg[:], data_ap=NEG16[:], idxs_ap=offT16[:],
            channels=128, num_elems=K, num_idxs=16)
        bb = lp.tile([128, K], bf, tag="bb")
        sc_i3 = nc.gpsimd.local_scatter(
            out_ap=bb[:], data_ap=ONE16[:], idxs_ap=offT16[:],
            channels=128, num_elems=K, num_idxs=16)
        bbinv = keep.tile([128, K], bf, tag="bbinv")
        nc.vector.tensor_scalar(out=bbinv[:], in0=bb[:], scalar1=-1.0, scalar2=1.0,
                                op0=Alu.mult, op1=Alu.add)
        if b == 0:
            dbg_dump("d_O", O[:])
            dbg_dump("d_Cc", Cc[:])
            dbg_dump("d_posm", posm[:])
            dbg_dump("d_ev", ev[:])
            dbg_dump("d_idx16", idx16[:])
            dbg_dump("d_inv", inv[:])
            dbg_dump("d_offT16", offT16[:])
            dbg_dump("d_bneg", bneg[:])
            dbg_dump("d_bbinv", bbinv[:])
            dbg_dump("d_endsc", endsc[:])
            dbg_dump("d_ribc", ribc[:])
            dbg_dump("d_eidxB", eidx[:])
        invf = lp.tile([128, K // 16], f32, tag="invf")
        nc.scalar.activation(out=invf[:], in_=inv[:], func=Act.Copy)
        # rhs8[q, (r,s)] = invf[q,s] * (group(q)==r); A3 matmul then yields
        # invall[q', (r,s)] = invf[16r + w(q'), s] = row-r's wrapped inverse
        # replicated to every core group.
        rhs8 = lp.tile([128, BLK * (K // 16)], f32, tag="rhs8")
        for rr_ in range(BLK):
            nc.vector.tensor_tensor(
                out=rhs8[:, rr_ * (K // 16):(rr_ + 1) * (K // 16)],
                in0=invf[:], in1=GR8[:, rr_:rr_ + 1].to_broadcast([128, K // 16]),
                op=Alu.mult)
        invall_ps = ivpool.tile([128, BLK * (K // 16)], f32, tag="invall_ps")
        nc.tensor.matmul(invall_ps[:], lhsT=A3[:], rhs=rhs8[:], start=True, stop=True)
        invall16 = keep.tile([128, BLK * (K // 16)], i16, tag="invall16")
        nc.scalar.activation(out=invall16[:], in_=invall_ps[:], func=Act.Copy)
        for sc in (sc_i1, sc_i2, sc_i3):
            if epoch_last_gather[0] is not None:
                add_dep_helper(sc.ins, epoch_last_gather[0].ins, False,
                               "pool library epoch order")
        scatter_insts.extend([sc_i1, sc_i2, sc_i3])
        blocks[b] = dict(invall16=invall16, eidx=eidx, ribc=ribc,
                         bneg=bneg, bbinv=bbinv)

    # ---------------- phase 2: value pipeline per row ----------------
    feats_tiles = {}
    out_tiles = {}
    scano_blk = [None, None]

    def value_row(g_row):
        ni, si = g_row // S, g_row % S
        b, r = g_row // BLK, g_row % BLK
        bk = blocks[b]

        half = (ni, si // SH)
        if half not in feats_tiles:
            s0 = (si // SH) * SH
            ft = fpool.tile([128, SH * K], f32, tag="ft")
            nc.sync.dma_start(
                out=ft[:],
                in_=feats_d[ni, :, s0:s0 + SH, :].rearrange("c s k -> c (s k)"))
            feats_tiles[half] = ft
        ft = feats_tiles[half]
        fs = (si % SH) * K

        invr16 = bk["invall16"][:, r * (K // 16):(r + 1) * (K // 16)]
        gath = vp.tile([128, K], f32, tag="gath")
        if os.environ.get("KPROBE") == "nogath":
            nc.vector.tensor_copy(out=gath[:], in_=ft[:, fs:fs + K])
        else:
            g_i = nc.gpsimd.ap_gather(
                out_ap=gath[:], in_ap=ft[:, fs:fs + K], idxs_ap=invr16,
                channels=128, num_elems=K, d=1, num_idxs=K)
            if scatter_insts:
                add_dep_helper(g_i.ins, scatter_insts[-1].ins, False,
                               "pool library phase order")

        # boundary rows (PSUM f32 via bf16 broadcast matmuls)
        brow = brpool.tile([128, 2 * K], f32, tag="brow")
        if os.environ.get("KPROBE") != "nobrow":
            nc.tensor.matmul(brow[:, 0:K], lhsT=EErb[r][:], rhs=bk["bneg"][:],
                             start=True, stop=True)
            nc.tensor.matmul(brow[:, K:2 * K], lhsT=EErb[r][:], rhs=bk["bbinv"][:],
                             start=True, stop=True)
        else:
            nc.tensor.matmul(brow[:], lhsT=EErb[r][:],
                             rhs=bk["bneg"][:, 0:K].to_broadcast([128, 2 * K]),
                             start=True, stop=True)

        if r % 4 == 0:
            sc_new = scpool.tile([128, 4 * 2 * K], f32, tag="scano")
            scano_blk[r // 4] = sc_new
        scano = scano_blk[r // 4]
        so = (r % 4) * 2 * K
        if os.environ.get("KPROBE") == "noscan":
            nc.vector.tensor_tensor(out=scano[:, so:so + K], in0=brow[:, 0:K],
                                    in1=gath[:], op=Alu.add)
            nc.vector.tensor_tensor(out=scano[:, so + K:so + 2 * K],
                                    in0=brow[:, K:2 * K], in1=gath[:], op=Alu.add)
        else:
            nc.vector.tensor_tensor_scan(
                out=scano[:, so:so + K], data0=brow[:, 0:K], data1=gath[:], initial=0.0,
                op0=Alu.add, op1=Alu.max)
            nc.vector.tensor_tensor_scan(
                out=scano[:, so + K:so + 2 * K], data0=brow[:, K:2 * K], data1=gath[:],
                initial=0.0, op0=Alu.mult, op1=Alu.add)

        if g_row == 0:
            dbg_dump("d_gath", gath[:])
            dbg_dump("d_scano", scano[:, 0:2 * K])

        if r == BLK - 1:
            # gather all 8 rows' segment-end values in one shot: [c, (r, m, p)]
            gath2 = vp.tile([128, 256], f32, tag="gath2")
            if os.environ.get("KPROBE") == "noends":
                nc.vector.tensor_copy(out=gath2[:], in_=scano_blk[0][:, 0:256])
            else:
              for hb in range(2):
                g2_i = nc.gpsimd.ap_gather(
                    out_ap=gath2[:, hb * 128:(hb + 1) * 128],
                    in_ap=scano_blk[hb][:],
                    idxs_ap=bk["eidx"][:, hb * 8:(hb + 1) * 8],
                    channels=128, num_elems=4 * 2 * K, d=1, num_idxs=128)
                if scatter_insts:
                    add_dep_helper(g2_i.ins, scatter_insts[-1].ins, False,
                                   "pool library phase order")  # noqa
                epoch_last_gather[0] = g2_i
            g2v = gath2[:].rearrange("c (s m p) -> c s m p", m=2, p=P)
            t1 = vp.tile([128, 128], f32, tag="t1")
            nc.vector.tensor_tensor(out=t1[:].rearrange("c (s p) -> c s p", p=P),
                                    in0=g2v[:, :, 1, :],
                                    in1=bk["ribc"][:, 0:128].rearrange("c (s p) -> c s p", p=P),
                                    op=Alu.mult)
            t2 = vp.tile([128, 128], f32, tag="t2")
            nc.vector.tensor_tensor(out=t2[:].rearrange("c (s p) -> c s p", p=P),
                                    in0=g2v[:, :, 0, :],
                                    in1=bk["ribc"][:, 128:256].rearrange("c (s p) -> c s p", p=P),
                                    op=Alu.mult)
            if g_row == BLK - 1:
                dbg_dump("d_gath2", gath2[:])
                dbg_dump("d_t1", t1[:, 0:P])
                dbg_dump("d_t2", t2[:, 0:P])
            # write combined rows into out accumulators (split at n boundary)
            row0 = b * BLK
            r_off = 0
            while r_off < BLK:
                gr = row0 + r_off
                ni2, si2 = gr // S, gr % S
                span = min(BLK - r_off, S - si2)
                if ni2 not in out_tiles:
                    ot_n = opool.tile([128, S * P], f32, tag="ot")
                    out_tiles[ni2] = ot_n
                ot2 = out_tiles[ni2]
                nc.vector.tensor_tensor(
                    out=ot2[:, si2 * P:(si2 + span) * P],
                    in0=t1[:, r_off * P:(r_off + span) * P],
                    in1=t2[:, r_off * P:(r_off + span) * P], op=Alu.add)
                if si2 + span == S:
                    nc.sync.dma_start(out=out_d[ni2].rearrange("c s p -> c (s p)"),
                                      in_=ot2[:])
                r_off += span


    # ---------------- epoch driver: overlap label and value phases ----------------
    only_p1 = os.environ.get("KPROBE") == "p1"
    EPOCHS = [(range(0, 8), range(0, 64)), (range(8, NBLK), range(64, ROWS))]
    for eblocks, erows in EPOCHS:
        for b_ in eblocks:
            label_block(b_)
        if not only_p1:
            for g_ in erows:
                value_row(g_)


def build_nc():
    if "nc" in _CACHE:
        return _CACHE["nc"]
    from concourse import bacc, mybir, tile
    dt = mybir.dt
    cn = _consts()
    nc = bacc.Bacc("TRN2", target_bir_lowering=False, debug=False,
                   enable_asserts=False, num_devices=N_CORES)
    dram = {}
    dram["feats"] = nc.dram_tensor("feats", [N_PER_CORE, C, S, K], dt.float32,
                                   kind="ExternalInput").ap()
    dram["labels"] = nc.dram_tensor("labels", [ROWS, K], dt.float32,
                                    kind="ExternalInput").ap()
    dram["out"] = nc.dram_tensor("out", [N_PER_CORE, C, S, P], dt.float32,
                                 kind="ExternalOutput").ap()

    def dtf(a):
        if a.dtype == np.int16:
            return dt.int16
        if str(a.dtype) == "bfloat16":
            return dt.bfloat16
        return dt.float32

    for k, v in cn.items():
        dram[k] = nc.dram_tensor(f"c_{k}", list(v.shape), dtf(v),
                                 kind="ExternalInput").ap()

    if os.environ.get("KDEBUG"):
        dbg_specs = {
            "d_O": ([128, K], dt.float32), "d_Cc": ([128, K], dt.float32),
            "d_posm": ([128, K], dt.float32), "d_ev": ([128, K], dt.float32),
            "d_idx16": ([128, K], dt.int16), "d_inv": ([128, K // 16], dt.int16),
            "d_offT16": ([128, 16], dt.int16), "d_bneg": ([128, K], dt.bfloat16),
            "d_bbinv": ([128, K], dt.bfloat16), "d_endsc": ([128, 1], dt.float32),
            "d_ribc": ([128, 256], dt.float32), "d_eidxB": ([128, 16], dt.int16),
            "d_invr16": ([128, K // 16], dt.int16), "d_gath": ([128, K], dt.float32),
            "d_scano": ([128, 2 * K], dt.float32),
            "d_gath2": ([128, 256], dt.float32),
            "d_t1": ([128, P], dt.float32), "d_t2": ([128, P], dt.float32),
        }
        for k, (shp, d) in dbg_specs.items():
            dram[k] = nc.dram_tensor(k, shp, d, kind="ExternalOutput").ap()

    with tile.TileContext(nc) as tc:
        with ExitStack() as stk:
            build_kernel_body(stk, tc, nc, dram)
    nc.compile()
    _CACHE["nc"] = nc
    _CACHE["consts"] = cn
    return nc


def _host_fallback(feats, part_labels, valid_mask, parts_num):
    n, c, s, k = feats.shape
    Pn = int(parts_num)
    f = np.asarray(feats, np.float32).transpose(0, 2, 3, 1).reshape(-1, c)
    seg = (np.asarray(part_labels).astype(np.int64).reshape(n * s, k)
           + np.arange(n * s, dtype=np.int64)[:, None] * Pn).reshape(-1)
    vm = np.asarray(valid_mask).reshape(-1).astype(np.float32)
    nsg = n * s * Pn
    psum = np.zeros((nsg, c), np.float32)
    np.add.at(psum, seg, f * vm[:, None])
    pcnt = np.zeros(nsg, np.float32)
    np.add.at(pcnt, seg, vm)
    patch = np.zeros(nsg, np.float32)
    np.add.at(patch, seg, np.ones_like(vm))
    smax = np.full((nsg, c), -np.inf, np.float32)
    np.maximum.at(smax, seg, f)
    pmax = np.where(patch[:, None] > 0, np.maximum(smax, -100.0), 0.0)
    pooled = psum / np.maximum(pcnt, 1.0)[:, None] + pmax
    return pooled.reshape(n, s, Pn, c).transpose(0, 3, 1, 2).astype(np.float32)


def kernel(feats, part_labels, valid_mask, parts_num):
    feats = np.ascontiguousarray(np.asarray(feats), dtype=np.float32)
    if int(parts_num) != P or feats.shape != (N, C, S, K) \
            or not bool(np.all(np.asarray(valid_mask))):
        return _host_fallback(feats, part_labels, valid_mask, parts_num)

    from concourse import bass_utils
    nc = build_nc()
    cn = _CACHE["consts"]
    labels_f32 = np.asarray(part_labels).astype(np.float32)

    in_maps = []
    for core in range(N_CORES):
        sl = slice(core * N_PER_CORE, (core + 1) * N_PER_CORE)
        m = {"feats": np.ascontiguousarray(feats[sl]),
             "labels": np.ascontiguousarray(labels_f32[sl]).reshape(ROWS, K)}
        for k, v in cn.items():
            m[f"c_{k}"] = v
        in_maps.append(m)

    res = bass_utils.run_bass_kernel_spmd(nc, in_maps, core_ids=list(range(N_CORES)))
    out = np.empty((N, C, S, P), np.float32)
    for core in range(N_CORES):
        out[core * N_PER_CORE:(core + 1) * N_PER_CORE] = res.results[core]["out"]
    return out



# revision 28
# speedup vs baseline: 1.5336x; 1.5336x over previous
"""Trainium2 Bass kernel: segment mean+max pooling (AnchorHeightPart).

Algorithm (per core, data-parallel over n: 4 n-batches/core):
  Host pre-sorts nothing value-wise; it builds, from the labels only, a
  counting-sort index table per (n,s) row with per-part counts padded up to
  multiples of 4 (hard bound 512 + 16*3 = 560 slots/row), pads pointing at an
  appended zero row. Values are biased +8 and cast fp16 so all real values are
  positive and zero-pads are neutral for both max and sum.

  Device: one transpose-mode dma_gather per n delivers the values sorted,
  padded, in [c, slot] layout straight from DRAM (DMA does load+permute in a
  single pass). Per row: PE broadcasts a 0/1 segment-reset mask (built by one
  gpsimd local_scatter per 6-row block), DVE runs a masked max-scan and a
  plain cumsum, Act downsamples both at the 4-grid (segment ends land on the
  grid by construction). Per block: one gpsimd ap_gather pulls both streams'
  segment-end values; five small DVE ops combine mean+max into the output.
"""

import os
import sys
from contextlib import ExitStack

import numpy as np

_REPO = "/opt/trn_rl_repo"
if _REPO not in sys.path and os.path.isdir(_REPO):
    sys.path.insert(0, _REPO)

N, C, S, K = 32, 128, 30, 512
P = 16
N_CORES = 8
N_PER_CORE = N // N_CORES          # 4
WPAD = 560                         # padded row width (hard max 512+16*3)
GRID = WPAD // 4                   # 140 grid slots per row
RB = 6                             # rows per block (6 of 8 lane groups used)
BPN = S // RB                      # 5 blocks per n
NBLK = N_PER_CORE * BPN            # 20 blocks per core
ZROW = S * K                       # zero-row index within an n's feats_t
NI = S * WPAD                      # 16800 gather idxs per n
NIP = 16896                        # padded to multiple of 128
BIAS = 8.0
GC = 512                           # idxs per dma_gather (HW transpose cap)
NCH = NIP // GC                    # 33 chunks per n
GIDX_COLS = 4224
SORTW = NIP

_CACHE = {}


def _consts():
    import ml_dtypes
    bf16 = ml_dtypes.bfloat16
    fp16 = np.float16
    q = np.arange(128)
    c = {}
    for r in range(RB):
        c[f"EErb{r}"] = (q[:, None] == 16 * r + q[None, :] % 16).astype(bf16)
    pm = np.ones((128, RB * P), fp16)
    pm[:, 0] = 0.0
    pm[:, 3 * P] = 0.0
    c["PM"] = pm
    c["ONE16"] = np.ones((128, 16), bf16)
    c["ZD0"] = np.zeros((128, 3 * WPAD), np.float32)
    return c


def _host_tables(lab):
    """lab: [N, S, K] int64 labels. Returns global table arrays."""
    oh = lab[..., None] == np.arange(P)
    counts = oh.sum(2)                            # [N,S,P]
    cntp = ((counts + 3) // 4) * 4
    offp = np.cumsum(cntp, axis=2) - cntp
    endp = offp + cntp - 1                        # == offp-1 when cntp == 0
    assert (offp[..., -1] + cntp[..., -1]).max() <= WPAD

    order = np.argsort(lab, axis=2, kind="stable")
    sortedlab = np.take_along_axis(lab, order, 2)
    cumx = np.cumsum(counts, axis=2) - counts
    rank = np.arange(K)[None, None, :] - np.take_along_axis(cumx, sortedlab, 2)
    slot = np.take_along_axis(offp, sortedlab, 2) + rank
    idxg = np.full((N, S, WPAD), ZROW, np.int64)
    np.put_along_axis(idxg, slot, order, axis=2)
    real = idxg != ZROW
    idxg = np.where(real, idxg + np.arange(S)[None, :, None] * K, ZROW)

    indic = (counts > 0).astype(np.float16)
    recip = np.where(counts > 0, 1.0 / np.maximum(counts, 1), 0.0).astype(np.float16)
    return dict(cntp=cntp, offp=offp, endp=endp, idxg=idxg,
                indic=indic, recip=recip)


def _core_tables(T, core):
    """Per-core DMA-ready tables."""
    n0 = core * N_PER_CORE
    # gather idx, wrapped i16, packed [128, 4*NIP//16].
    # n0 is split into a 1-block head (rows 0-5, 3456 idxs) + tail (rows
    # 6-29, 13440 idxs) so compute can start before the full gather lands.
    def wrap(flat, pad_to):
        flat = np.concatenate([flat, np.full(pad_to - len(flat), ZROW, np.int64)])
        w = flat.reshape(pad_to // 16, 16).T.astype(np.int16)
        return np.tile(w, (8, 1))                              # [128, pad/16]

    parts = [wrap(T["idxg"][n0 + ni].reshape(-1), NIP)
             for ni in range(N_PER_CORE)]
    gidx = np.concatenate(parts, axis=1)
    assert gidx.shape[1] == GIDX_COLS
    # per-block tables
    offidx = np.full((NBLK, 128, 16), -1, np.int16)
    endsidx = np.empty((NBLK, 128, 12), np.int16)
    ctab = np.empty((NBLK, 128, 2 * RB * P), np.float16)
    for b in range(NBLK):
        ni, bi = b // BPN, b % BPN
        n = n0 + ni
        for r in range(RB):
            s = bi * RB + r
            op_ = T["offp"][n, s]
            cp_ = T["cntp"][n, s]
            row = np.where(cp_ > 0, op_, -1).astype(np.int16)
            offidx[b, 16 * r:16 * (r + 1), :] = row[None, :]
        # ends idx: j = m*96 + r*16 + p -> r*280 + m*140 + max(endp//4, 0)
        vals = np.empty(2 * RB * P, np.int64)
        for m in range(2):
            for r in range(RB):
                s = bi * RB + r
                g = np.maximum(T["endp"][n, s] // 4, 0)
                vals[m * RB * P + r * P:(m * RB * P + r * P) + P] = \
                    (r // 3) * 6 * GRID + m * 3 * GRID + (r % 3) * GRID + g
        w = vals.reshape(12, 16).T.astype(np.int16)            # [16, 12]
        endsidx[b] = np.tile(w, (8, 1))
        ct = np.empty(2 * RB * P, np.float16)
        for r in range(RB):
            s = bi * RB + r
            ct[r * P:(r + 1) * P] = T["indic"][n, s]
            ct[RB * P + r * P:RB * P + (r + 1) * P] = T["recip"][n, s]
        ctab[b] = np.broadcast_to(ct, (128, 2 * RB * P))
    # pre-transpose to [128, NBLK*X] so the preload DMAs are contiguous
    offidx_t = np.ascontiguousarray(offidx.transpose(1, 0, 2).reshape(128, -1))
    endsidx_t = np.ascontiguousarray(endsidx.transpose(1, 0, 2).reshape(128, -1))
    ctab_t = np.ascontiguousarray(ctab.transpose(1, 0, 2).reshape(128, -1))
    return dict(gidx=gidx, offidx=offidx_t, endsidx=endsidx_t, ctab=ctab_t)


def build_kernel_body(stk, tc, nc, dram):
    from concourse import mybir
    from concourse.tile_rust import add_dep_helper
    dt = mybir.dt
    Alu = mybir.AluOpType
    Act = mybir.ActivationFunctionType
    f32, i16, h16, bf = dt.float32, dt.int16, dt.float16, dt.bfloat16

    feats_d = dram["feats"]       # [4, ZROW+2, C] fp16 (biased, zero rows)
    gidx_d = dram["gidx"]         # [4, 128, NIP//16] i16
    offidx_d = dram["offidx"]     # [NBLK, 128, 16] i16
    endsidx_d = dram["endsidx"]   # [NBLK, 128, 12] i16
    ctab_d = dram["ctab"]         # [NBLK, 128, 192] fp16
    out_d = dram["out"]           # [4, C, S, P] f32

    cpool = stk.enter_context(tc.tile_pool(name="consts", bufs=1))
    spool = stk.enter_context(tc.tile_pool(name="sorted", bufs=2))
    ipool = stk.enter_context(tc.tile_pool(name="idx", bufs=2))
    bpool = stk.enter_context(tc.tile_pool(name="bb", bufs=4))
    tpool = stk.enter_context(tc.tile_pool(name="tabs", bufs=3))
    ppool = stk.enter_context(tc.tile_pool(name="brow", bufs=2, space="PSUM"))
    mpool = stk.enter_context(tc.tile_pool(name="scan", bufs=3))
    dpool = stk.enter_context(tc.tile_pool(name="down", bufs=3))
    gpool = stk.enter_context(tc.tile_pool(name="ends", bufs=3))
    opool = stk.enter_context(tc.tile_pool(name="oacc", bufs=2))

    def ldconst(name, dtype=f32):
        a = dram[name]
        t = cpool.tile(list(a.shape), dtype, tag=name)
        nc.sync.dma_start(out=t[:], in_=a[:])
        return t

    # preload everything up front, most-urgent first, so no DMA queues
    # behind the big feats gathers during steady state
    gidx_all = cpool.tile([128, GIDX_COLS], i16, tag="gidx_all")
    nc.sync.dma_start(out=gidx_all[:, 0:432], in_=gidx_d[:, 0:432])
    nc.sync.dma_start(out=gidx_all[:, 432:GIDX_COLS], in_=gidx_d[:, 432:GIDX_COLS])
    ONE16 = ldconst("ONE16", dtype=bf)
    off_all = cpool.tile([128, NBLK * 16], i16, tag="off_all")
    nc.sync.dma_start(out=off_all[:], in_=offidx_d[:])
    EErb = [ldconst(f"EErb{r}", dtype=bf) for r in range(RB)]
    PM = ldconst("PM", dtype=h16)
    ZD0 = ldconst("ZD0", dtype=f32)
    ends_all = cpool.tile([128, NBLK * 12], i16, tag="ends_all")
    nc.sync.dma_start(out=ends_all[:], in_=endsidx_d[:])
    ctab_all = cpool.tile([128, NBLK * 2 * RB * P], h16, tag="ctab_all")
    nc.sync.dma_start(out=ctab_all[:], in_=ctab_d[:])

    last_pool_op = [None]

    def chain_pool(inst):
        if last_pool_op[0] is not None:
            add_dep_helper(inst.ins, last_pool_op[0].ins, False,
                           "pool library phase order")
        last_pool_op[0] = inst

    sortv_t = {}
    oacc_t = {}
    bbinv_t = {}
    ends_t = {}

    next_chunk = {}

    def issue_chunks(ni, upto):
        """Issue 512-idx gather chunks for n=ni until `upto` chunks done."""
        if ni not in sortv_t:
            sortv_new = spool.tile([128, SORTW], h16, tag="sortv")
            sortv_t[ni] = sortv_new
            next_chunk[ni] = 0
        sortv = sortv_t[ni]
        upto = min(upto, NCH)
        for ch in range(next_chunk[ni], upto):
            o0 = ch * GC
            g_i = nc.gpsimd.dma_gather(
                out_ap=sortv[:, o0:o0 + GC].rearrange("c (o n) -> c o n", o=1),
                in_ap=feats_d[ni],
                idxs_ap=gidx_all[:, ni * (NIP // 16) + ch * (GC // 16):
                                 ni * (NIP // 16) + (ch + 1) * (GC // 16)],
                num_idxs=GC, num_idxs_reg=GC,
                elem_size=C, transpose=True)
            chain_pool(g_i)
        next_chunk[ni] = max(next_chunk[ni], upto)

    def issue_mask(b):
        bb = bpool.tile([128, WPAD], bf, tag="bbt")
        sc_i = nc.gpsimd.local_scatter(
            out_ap=bb[:], data_ap=ONE16[:], idxs_ap=off_all[:, b * 16:(b + 1) * 16],
            channels=128, num_elems=WPAD, num_idxs=16)
        chain_pool(sc_i)
        bbinv = bpool.tile([128, WPAD], bf, tag="bbinvt")
        nc.scalar.activation(out=bbinv[:], in_=bb[:], func=Act.Identity,
                             bias=1.0, scale=-1.0)
        bbinv_t[b] = bbinv

    def issue_rows_and_ends(b):
        ni, bi = b // BPN, b % BPN
        sortv = sortv_t[ni]
        bbinv = bbinv_t.pop(b)
        endt = ends_all[:, b * 12:(b + 1) * 12]
        D = dpool.tile([128, RB * 2 * GRID], f32, tag="D")
        W3 = 3 * WPAD
        for g3 in range(2):
            # [128, 2048] f32 = exactly 4 PSUM banks, so 2 bufs fill PSUM and
            # every tile is bank-aligned; matmul writes must not cross the
            # 512-col bank lines, so split each row's mask at them.
            brow = ppool.tile([128, 2048], f32, tag="brow")
            for rr in range(3):
                r = g3 * 3 + rr
                a, b_ = rr * WPAD, (rr + 1) * WPAD
                cut = ((a // 512) + 1) * 512
                nc.tensor.matmul(brow[:, a:cut], lhsT=EErb[r][:],
                                 rhs=bbinv[:, 0:cut - a],
                                 start=True, stop=True)
                nc.tensor.matmul(brow[:, cut:b_], lhsT=EErb[r][:],
                                 rhs=bbinv[:, cut - a:WPAD],
                                 start=True, stop=True)
            s = bi * RB + g3 * 3
            xo = s * WPAD
            xsl = sortv[:, xo:xo + W3]
            maxo = mpool.tile([128, W3], h16, tag="maxo")
            nc.vector.tensor_tensor_scan(
                out=maxo[:], data0=brow[:, 0:W3], data1=xsl, initial=0.0,
                op0=Alu.mult, op1=Alu.max)
            sumo = mpool.tile([128, W3], f32, tag="sumo")
            nc.vector.tensor_tensor_scan(
                out=sumo[:], data0=ZD0[:], data1=xsl, initial=0.0,
                op0=Alu.add, op1=Alu.add)
            d0 = g3 * 6 * GRID
            mview = maxo[:].rearrange("c (g f) -> c g f", f=4)[:, :, 3]
            nc.scalar.copy(out=D[:, d0:d0 + 3 * GRID], in_=mview)
            sview = sumo[:].rearrange("c (g f) -> c g f", f=4)[:, :, 3]
            nc.scalar.copy(out=D[:, d0 + 3 * GRID:d0 + 6 * GRID], in_=sview)
        Gt = gpool.tile([128, 2 * RB * P], f32, tag="Gt")
        g2_i = nc.gpsimd.ap_gather(
            out_ap=Gt[:], in_ap=D[:], idxs_ap=endt,
            channels=128, num_elems=RB * 2 * GRID, d=1, num_idxs=2 * RB * P)
        chain_pool(g2_i)
        ends_t[b] = Gt

    def issue_combine(b):
        ni, bi = b // BPN, b % BPN
        Gt = ends_t.pop(b)
        ctt = ctab_all[:, b * 2 * RB * P:(b + 1) * 2 * RB * P]
        if bi == 0:
            oacc_new = opool.tile([128, S * P], f32, tag="oacc")
            oacc_t[ni] = oacc_new
        oacc = oacc_t[ni]
        NP = RB * P  # 96
        u = gpool.tile([128, NP], f32, tag="u")
        nc.vector.scalar_tensor_tensor(
            out=u[:], in0=Gt[:, 0:NP], scalar=-2.0 * BIAS,
            in1=ctt[:, 0:NP], op0=Alu.add, op1=Alu.mult)
        v = gpool.tile([128, NP], f32, tag="v")
        nc.vector.tensor_tensor(out=v[:], in0=Gt[:, NP - 1:2 * NP - 1],
                                in1=PM[:], op=Alu.mult)
        w = gpool.tile([128, NP], f32, tag="w")
        nc.vector.tensor_tensor(out=w[:], in0=Gt[:, NP:2 * NP],
                                in1=v[:], op=Alu.subtract)
        x2 = gpool.tile([128, NP], f32, tag="x2")
        nc.vector.tensor_tensor(out=x2[:], in0=w[:],
                                in1=ctt[:, NP:2 * NP], op=Alu.mult)
        nc.vector.tensor_tensor(out=oacc[:, bi * NP:(bi + 1) * NP],
                                in0=u[:], in1=x2[:], op=Alu.add)
        if bi == BPN - 1:
            nc.sync.dma_start(out=out_d[ni].rearrange("c s p -> c (s p)"),
                              in_=oacc_t.pop(ni)[:])

    # software-pipelined schedule: masks two blocks ahead; gather chunks
    # issued with one-block lookahead, next n's chunks trickled in early
    def cover(bi):
        return -(-((bi + 1) * RB * WPAD) // GC)   # chunks covering block bi

    issue_chunks(0, cover(0))
    issue_mask(0)
    issue_mask(1)
    for b in range(NBLK):
        ni, bi = b // BPN, b % BPN
        if b + 2 < NBLK:
            issue_mask(b + 2)
        issue_chunks(ni, cover(bi + 1))
        if bi >= 1 and ni + 1 < N_PER_CORE:
            issue_chunks(ni + 1, bi * 9)
        issue_rows_and_ends(b)
        if b >= 1:
            issue_combine(b - 1)
    issue_combine(NBLK - 1)


def build_nc():
    if "nc" in _CACHE:
        return _CACHE["nc"]
    from concourse import bacc, mybir, tile
    dt = mybir.dt
    cn = _consts()
    nc = bacc.Bacc("TRN2", target_bir_lowering=False, debug=False,
                   enable_asserts=False, num_devices=N_CORES,
                   dynamic_dma_scratch_size=32768)
    dram = {}
    dram["feats"] = nc.dram_tensor("feats", [N_PER_CORE, ZROW + 2, C],
                                   dt.float16, kind="ExternalInput").ap()
    dram["gidx"] = nc.dram_tensor("gidx", [128, GIDX_COLS],
                                  dt.int16, kind="ExternalInput").ap()
    dram["offidx"] = nc.dram_tensor("offidx", [128, NBLK * 16], dt.int16,
                                    kind="ExternalInput").ap()
    dram["endsidx"] = nc.dram_tensor("endsidx", [128, NBLK * 12], dt.int16,
                                     kind="ExternalInput").ap()
    dram["ctab"] = nc.dram_tensor("ctab", [128, NBLK * 2 * RB * P], dt.float16,
                                  kind="ExternalInput").ap()
    dram["out"] = nc.dram_tensor("out", [N_PER_CORE, C, S, P], dt.float32,
                                 kind="ExternalOutput").ap()

    def dtf(a):
        if a.dtype == np.int16:
            return dt.int16
        n = str(a.dtype)
        if n == "bfloat16":
            return dt.bfloat16
        if n == "float16":
            return dt.float16
        return dt.float32

    for k, v in cn.items():
        dram[k] = nc.dram_tensor(f"c_{k}", list(v.shape), dtf(v),
                                 kind="ExternalInput").ap()

    with tile.TileContext(nc) as tc:
        with ExitStack() as stk:
            build_kernel_body(stk, tc, nc, dram)
    nc.compile()
    _CACHE["nc"] = nc
    _CACHE["consts"] = cn
    return nc


def _host_fallback(feats, part_labels, valid_mask, parts_num):
    n, c, s, k = feats.shape
    Pn = int(parts_num)
    f = np.asarray(feats, np.float32).transpose(0, 2, 3, 1).reshape(-1, c)
    seg = (np.asarray(part_labels).astype(np.int64).reshape(n * s, k)
           + np.arange(n * s, dtype=np.int64)[:, None] * Pn).reshape(-1)
    vm = np.asarray(valid_mask).reshape(-1).astype(np.float32)
    nsg = n * s * Pn
    psum = np.zeros((nsg, c), np.float32)
    np.add.at(psum, seg, f * vm[:, None])
    pcnt = np.zeros(nsg, np.float32)
    np.add.at(pcnt, seg, vm)
    patch = np.zeros(nsg, np.float32)
    np.add.at(patch, seg, np.ones_like(vm))
    smax = np.full((nsg, c), -np.inf, np.float32)
    np.maximum.at(smax, seg, f)
    pmax = np.where(patch[:, None] > 0, np.maximum(smax, -100.0), 0.0)
    pooled = psum / np.maximum(pcnt, 1.0)[:, None] + pmax
    return pooled.reshape(n, s, Pn, c).transpose(0, 3, 1, 2).astype(np.float32)


def kernel(feats, part_labels, valid_mask, parts_num):
    feats = np.ascontiguousarray(np.asarray(feats), dtype=np.float32)
    if int(parts_num) != P or feats.shape != (N, C, S, K) \
            or not bool(np.all(np.asarray(valid_mask))):
        return _host_fallback(feats, part_labels, valid_mask, parts_num)

    from concourse import bass_utils
    nc = build_nc()
    cn = _CACHE["consts"]

    lab = np.asarray(part_labels).astype(np.int64)
    if int(lab.min()) < 0 or int(lab.max()) >= P:
        return _host_fallback(feats, part_labels, valid_mask, parts_num)
    T = _host_tables(lab)
    # the Sdiff neighbor-shift needs a valid prefix-sum baseline in every
    # row's part-0 grid slot; an empty part 0 would corrupt part 1's mean
    if int(T["cntp"].min()) == 0:
        return _host_fallback(feats, part_labels, valid_mask, parts_num)
    # feats_t: [N, S*K+2, C] fp16, biased, zero rows appended
    ft = feats.transpose(0, 2, 3, 1).reshape(N, S * K, C) + BIAS
    ft = np.concatenate([ft, np.zeros((N, 2, C), np.float32)], 1)
    ft = ft.astype(np.float16)

    in_maps = []
    for core in range(N_CORES):
        ct = _core_tables(T, core)
        sl = slice(core * N_PER_CORE, (core + 1) * N_PER_CORE)
        m = {"feats": np.ascontiguousarray(ft[sl]),
             "gidx": ct["gidx"], "offidx": ct["offidx"],
             "endsidx": ct["endsidx"], "ctab": ct["ctab"]}
        for k, v in cn.items():
            m[f"c_{k}"] = v
        in_maps.append(m)

    res = bass_utils.run_bass_kernel_spmd(nc, in_maps, core_ids=list(range(N_CORES)))
    out = np.empty((N, C, S, P), np.float32)
    for core in range(N_CORES):
        out[core * N_PER_CORE:(core + 1) * N_PER_CORE] = res.results[core]["out"]
    return out


# revision 29
# speedup vs baseline: 1.8169x; 1.1848x over previous
"""Trainium2 Bass kernel: segment mean+max pooling (AnchorHeightPart).

Algorithm (per core, data-parallel over n: 4 n-batches/core):
  Host pre-sorts nothing value-wise; it builds, from the labels only, a
  counting-sort index table per (n,s) row with per-part counts padded up to
  multiples of 4 (hard bound 512 + 16*3 = 560 slots/row), pads pointing at an
  appended zero row. Values are biased +8 and cast fp16 so all real values are
  positive and zero-pads are neutral for both max and sum.

  Device: one transpose-mode dma_gather per n delivers the values sorted,
  padded, in [c, slot] layout straight from DRAM (DMA does load+permute in a
  single pass). Per row: PE broadcasts a 0/1 segment-reset mask (built by one
  gpsimd local_scatter per 6-row block), DVE runs a masked max-scan and a
  plain cumsum, Act downsamples both at the 4-grid (segment ends land on the
  grid by construction). Per block: one gpsimd ap_gather pulls both streams'
  segment-end values; five small DVE ops combine mean+max into the output.
"""

import os
import sys
from contextlib import ExitStack

import numpy as np

_REPO = "/opt/trn_rl_repo"
if _REPO not in sys.path and os.path.isdir(_REPO):
    sys.path.insert(0, _REPO)

N, C, S, K = 32, 128, 30, 512
P = 16
N_CORES = 8
N_PER_CORE = N // N_CORES          # 4
WPAD = 560                         # padded row width (hard max 512+16*3)
GRID = WPAD // 4                   # 140 grid slots per row
RB = 6                             # rows per block (6 of 8 lane groups used)
BPN = S // RB                      # 5 blocks per n
NBLK = N_PER_CORE * BPN            # 20 blocks per core
ZROW = S * K                       # zero-row index within an n's feats_t
NI = S * WPAD                      # 16800 gather idxs per n
NIP = 16896                        # padded to multiple of 128
BIAS = 8.0
GC = 768                           # idxs per dma_gather (HW-verified; 1024 crashes)
NCH = NIP // GC                    # 33 chunks per n
GIDX_COLS = 4224
SORTW = NIP

_CACHE = {}


def _consts():
    import ml_dtypes
    bf16 = ml_dtypes.bfloat16
    fp16 = np.float16
    q = np.arange(128)
    c = {}
    for r in range(RB):
        c[f"EErb{r}"] = (q[:, None] == 16 * r + q[None, :] % 16).astype(bf16)
    pm = np.ones((128, RB * P), fp16)
    pm[:, 0] = 0.0
    pm[:, 3 * P] = 0.0
    c["PM"] = pm
    c["ONE16"] = np.ones((128, 16), bf16)
    c["ZD0"] = np.zeros((128, 3 * WPAD), np.float32)
    return c


def _host_tables(lab):
    """lab: [N, S, K] int64 labels. Returns global table arrays."""
    oh = lab[..., None] == np.arange(P)
    counts = oh.sum(2)                            # [N,S,P]
    cntp = ((counts + 3) // 4) * 4
    offp = np.cumsum(cntp, axis=2) - cntp
    endp = offp + cntp - 1                        # == offp-1 when cntp == 0
    assert (offp[..., -1] + cntp[..., -1]).max() <= WPAD

    order = np.argsort(lab, axis=2, kind="stable")
    sortedlab = np.take_along_axis(lab, order, 2)
    cumx = np.cumsum(counts, axis=2) - counts
    rank = np.arange(K)[None, None, :] - np.take_along_axis(cumx, sortedlab, 2)
    slot = np.take_along_axis(offp, sortedlab, 2) + rank
    idxg = np.full((N, S, WPAD), ZROW, np.int64)
    np.put_along_axis(idxg, slot, order, axis=2)
    real = idxg != ZROW
    idxg = np.where(real, idxg + np.arange(S)[None, :, None] * K, ZROW)

    indic = (counts > 0).astype(np.float16)
    recip = np.where(counts > 0, 1.0 / np.maximum(counts, 1), 0.0).astype(np.float16)
    return dict(cntp=cntp, offp=offp, endp=endp, idxg=idxg,
                indic=indic, recip=recip)


def _core_tables(T, core):
    """Per-core DMA-ready tables."""
    n0 = core * N_PER_CORE
    # gather idx, wrapped i16, packed [128, 4*NIP//16].
    # n0 is split into a 1-block head (rows 0-5, 3456 idxs) + tail (rows
    # 6-29, 13440 idxs) so compute can start before the full gather lands.
    def wrap(flat, pad_to):
        flat = np.concatenate([flat, np.full(pad_to - len(flat), ZROW, np.int64)])
        w = flat.reshape(pad_to // 16, 16).T.astype(np.int16)
        return np.tile(w, (8, 1))                              # [128, pad/16]

    parts = [wrap(T["idxg"][n0 + ni].reshape(-1), NIP)
             for ni in range(N_PER_CORE)]
    gidx = np.concatenate(parts, axis=1)
    assert gidx.shape[1] == GIDX_COLS
    # per-block tables
    offidx = np.full((NBLK, 128, 16), -1, np.int16)
    endsidx = np.empty((NBLK, 128, 12), np.int16)
    ctab = np.empty((NBLK, 128, 2 * RB * P), np.float16)
    for b in range(NBLK):
        ni, bi = b // BPN, b % BPN
        n = n0 + ni
        for r in range(RB):
            s = bi * RB + r
            op_ = T["offp"][n, s]
            cp_ = T["cntp"][n, s]
            row = np.where(cp_ > 0, op_, -1).astype(np.int16)
            offidx[b, 16 * r:16 * (r + 1), :] = row[None, :]
        # ends idx: j = m*96 + r*16 + p -> r*280 + m*140 + max(endp//4, 0)
        vals = np.empty(2 * RB * P, np.int64)
        for m in range(2):
            for r in range(RB):
                s = bi * RB + r
                g = np.maximum(T["endp"][n, s] // 4, 0)
                vals[m * RB * P + r * P:(m * RB * P + r * P) + P] = \
                    (r // 3) * 6 * GRID + m * 3 * GRID + (r % 3) * GRID + g
        w = vals.reshape(12, 16).T.astype(np.int16)            # [16, 12]
        endsidx[b] = np.tile(w, (8, 1))
        ct = np.empty(2 * RB * P, np.float16)
        for r in range(RB):
            s = bi * RB + r
            ct[r * P:(r + 1) * P] = T["indic"][n, s]
            ct[RB * P + r * P:RB * P + (r + 1) * P] = T["recip"][n, s]
        ctab[b] = np.broadcast_to(ct, (128, 2 * RB * P))
    # pre-transpose to [128, NBLK*X] so the preload DMAs are contiguous
    offidx_t = np.ascontiguousarray(offidx.transpose(1, 0, 2).reshape(128, -1))
    endsidx_t = np.ascontiguousarray(endsidx.transpose(1, 0, 2).reshape(128, -1))
    ctab_t = np.ascontiguousarray(ctab.transpose(1, 0, 2).reshape(128, -1))
    return dict(gidx=gidx, offidx=offidx_t, endsidx=endsidx_t, ctab=ctab_t)


def build_kernel_body(stk, tc, nc, dram):
    from concourse import mybir
    from concourse.tile_rust import add_dep_helper
    dt = mybir.dt
    Alu = mybir.AluOpType
    Act = mybir.ActivationFunctionType
    f32, i16, h16, bf = dt.float32, dt.int16, dt.float16, dt.bfloat16

    feats_d = dram["feats"]       # [4, ZROW+2, C] fp16 (biased, zero rows)
    gidx_d = dram["gidx"]         # [4, 128, NIP//16] i16
    offidx_d = dram["offidx"]     # [NBLK, 128, 16] i16
    endsidx_d = dram["endsidx"]   # [NBLK, 128, 12] i16
    ctab_d = dram["ctab"]         # [NBLK, 128, 192] fp16
    out_d = dram["out"]           # [4, C, S, P] f32

    cpool = stk.enter_context(tc.tile_pool(name="consts", bufs=1))
    spool = stk.enter_context(tc.tile_pool(name="sorted", bufs=2))
    ipool = stk.enter_context(tc.tile_pool(name="idx", bufs=2))
    bpool = stk.enter_context(tc.tile_pool(name="bb", bufs=4))
    tpool = stk.enter_context(tc.tile_pool(name="tabs", bufs=3))
    ppool = stk.enter_context(tc.tile_pool(name="brow", bufs=2, space="PSUM"))
    mpool = stk.enter_context(tc.tile_pool(name="scan", bufs=3))
    dpool = stk.enter_context(tc.tile_pool(name="down", bufs=3))
    gpool = stk.enter_context(tc.tile_pool(name="ends", bufs=3))
    opool = stk.enter_context(tc.tile_pool(name="oacc", bufs=2))

    def ldconst(name, dtype=f32):
        a = dram[name]
        t = cpool.tile(list(a.shape), dtype, tag=name)
        nc.sync.dma_start(out=t[:], in_=a[:])
        return t

    # preload everything up front, most-urgent first, so no DMA queues
    # behind the big feats gathers during steady state
    gidx_all = cpool.tile([128, GIDX_COLS], i16, tag="gidx_all")
    nc.sync.dma_start(out=gidx_all[:, 0:432], in_=gidx_d[:, 0:432])
    nc.sync.dma_start(out=gidx_all[:, 432:GIDX_COLS], in_=gidx_d[:, 432:GIDX_COLS])
    ONE16 = ldconst("ONE16", dtype=bf)
    off_all = cpool.tile([128, NBLK * 16], i16, tag="off_all")
    nc.sync.dma_start(out=off_all[:], in_=offidx_d[:])
    EErb = [ldconst(f"EErb{r}", dtype=bf) for r in range(RB)]
    PM = ldconst("PM", dtype=h16)
    ZD0 = ldconst("ZD0", dtype=f32)
    ends_all = cpool.tile([128, NBLK * 12], i16, tag="ends_all")
    nc.sync.dma_start(out=ends_all[:], in_=endsidx_d[:])
    ctab_all = cpool.tile([128, NBLK * 2 * RB * P], h16, tag="ctab_all")
    nc.sync.dma_start(out=ctab_all[:], in_=ctab_d[:])

    last_pool_op = [None]

    def chain_pool(inst):
        if last_pool_op[0] is not None:
            add_dep_helper(inst.ins, last_pool_op[0].ins, False,
                           "pool library phase order")
        last_pool_op[0] = inst

    sortv_t = {}
    oacc_t = {}
    bbinv_t = {}
    ends_t = {}

    next_chunk = {}

    def issue_chunks(ni, upto):
        """Issue 512-idx gather chunks for n=ni until `upto` chunks done."""
        if ni not in sortv_t:
            sortv_new = spool.tile([128, SORTW], h16, tag="sortv")
            sortv_t[ni] = sortv_new
            next_chunk[ni] = 0
        sortv = sortv_t[ni]
        upto = min(upto, NCH)
        for ch in range(next_chunk[ni], upto):
            o0 = ch * GC
            g_i = nc.gpsimd.dma_gather(
                out_ap=sortv[:, o0:o0 + GC].rearrange("c (o n) -> c o n", o=1),
                in_ap=feats_d[ni],
                idxs_ap=gidx_all[:, ni * (NIP // 16) + ch * (GC // 16):
                                 ni * (NIP // 16) + (ch + 1) * (GC // 16)],
                num_idxs=GC, num_idxs_reg=GC,
                elem_size=C, transpose=True)
            chain_pool(g_i)
        next_chunk[ni] = max(next_chunk[ni], upto)

    def issue_mask(b):
        bb = bpool.tile([128, WPAD], bf, tag="bbt")
        sc_i = nc.gpsimd.local_scatter(
            out_ap=bb[:], data_ap=ONE16[:], idxs_ap=off_all[:, b * 16:(b + 1) * 16],
            channels=128, num_elems=WPAD, num_idxs=16)
        chain_pool(sc_i)
        bbinv = bpool.tile([128, WPAD], bf, tag="bbinvt")
        nc.scalar.activation(out=bbinv[:], in_=bb[:], func=Act.Identity,
                             bias=1.0, scale=-1.0)
        bbinv_t[b] = bbinv

    def issue_rows_and_ends(b):
        ni, bi = b // BPN, b % BPN
        sortv = sortv_t[ni]
        bbinv = bbinv_t.pop(b)
        endt = ends_all[:, b * 12:(b + 1) * 12]
        D = dpool.tile([128, RB * 2 * GRID], f32, tag="D")
        W3 = 3 * WPAD
        for g3 in range(2):
            # [128, 2048] f32 = exactly 4 PSUM banks, so 2 bufs fill PSUM and
            # every tile is bank-aligned; matmul writes must not cross the
            # 512-col bank lines, so split each row's mask at them.
            brow = ppool.tile([128, 2048], f32, tag="brow")
            for rr in range(3):
                r = g3 * 3 + rr
                a, b_ = rr * WPAD, (rr + 1) * WPAD
                cut = ((a // 512) + 1) * 512
                nc.tensor.matmul(brow[:, a:cut], lhsT=EErb[r][:],
                                 rhs=bbinv[:, 0:cut - a],
                                 start=True, stop=True)
                nc.tensor.matmul(brow[:, cut:b_], lhsT=EErb[r][:],
                                 rhs=bbinv[:, cut - a:WPAD],
                                 start=True, stop=True)
            s = bi * RB + g3 * 3
            xo = s * WPAD
            xsl = sortv[:, xo:xo + W3]
            maxo = mpool.tile([128, W3], h16, tag="maxo")
            nc.vector.tensor_tensor_scan(
                out=maxo[:], data0=brow[:, 0:W3], data1=xsl, initial=0.0,
                op0=Alu.mult, op1=Alu.max)
            sumo = mpool.tile([128, W3], f32, tag="sumo")
            nc.vector.tensor_tensor_scan(
                out=sumo[:], data0=ZD0[:], data1=xsl, initial=0.0,
                op0=Alu.add, op1=Alu.add)
            d0 = g3 * 6 * GRID
            mview = maxo[:].rearrange("c (g f) -> c g f", f=4)[:, :, 3]
            nc.scalar.copy(out=D[:, d0:d0 + 3 * GRID], in_=mview)
            sview = sumo[:].rearrange("c (g f) -> c g f", f=4)[:, :, 3]
            nc.scalar.copy(out=D[:, d0 + 3 * GRID:d0 + 6 * GRID], in_=sview)
        Gt = gpool.tile([128, 2 * RB * P], f32, tag="Gt")
        g2_i = nc.gpsimd.ap_gather(
            out_ap=Gt[:], in_ap=D[:], idxs_ap=endt,
            channels=128, num_elems=RB * 2 * GRID, d=1, num_idxs=2 * RB * P)
        chain_pool(g2_i)
        ends_t[b] = Gt

    def issue_combine(b):
        ni, bi = b // BPN, b % BPN
        Gt = ends_t.pop(b)
        ctt = ctab_all[:, b * 2 * RB * P:(b + 1) * 2 * RB * P]
        if bi == 0:
            oacc_new = opool.tile([128, S * P], f32, tag="oacc")
            oacc_t[ni] = oacc_new
        oacc = oacc_t[ni]
        NP = RB * P  # 96
        u = gpool.tile([128, NP], f32, tag="u")
        nc.vector.scalar_tensor_tensor(
            out=u[:], in0=Gt[:, 0:NP], scalar=-2.0 * BIAS,
            in1=ctt[:, 0:NP], op0=Alu.add, op1=Alu.mult)
        v = gpool.tile([128, NP], f32, tag="v")
        nc.vector.tensor_tensor(out=v[:], in0=Gt[:, NP - 1:2 * NP - 1],
                                in1=PM[:], op=Alu.mult)
        w = gpool.tile([128, NP], f32, tag="w")
        nc.vector.tensor_tensor(out=w[:], in0=Gt[:, NP:2 * NP],
                                in1=v[:], op=Alu.subtract)
        x2 = gpool.tile([128, NP], f32, tag="x2")
        nc.vector.tensor_tensor(out=x2[:], in0=w[:],
                                in1=ctt[:, NP:2 * NP], op=Alu.mult)
        nc.vector.tensor_tensor(out=oacc[:, bi * NP:(bi + 1) * NP],
                                in0=u[:], in1=x2[:], op=Alu.add)
        if bi == BPN - 1:
            nc.sync.dma_start(out=out_d[ni].rearrange("c s p -> c (s p)"),
                              in_=oacc_t.pop(ni)[:])

    # software-pipelined schedule: masks two blocks ahead; gather chunks
    # issued with one-block lookahead, next n's chunks trickled in early
    def cover(bi):
        return -(-((bi + 1) * RB * WPAD) // GC)   # chunks covering block bi

    issue_chunks(0, cover(0))
    issue_mask(0)
    issue_mask(1)
    for b in range(NBLK):
        ni, bi = b // BPN, b % BPN
        if b + 2 < NBLK:
            issue_mask(b + 2)
        issue_chunks(ni, cover(bi + 1))
        if bi >= 1 and ni + 1 < N_PER_CORE:
            issue_chunks(ni + 1, bi * 6)
        issue_rows_and_ends(b)
        if b >= 1:
            issue_combine(b - 1)
    issue_combine(NBLK - 1)


def build_nc():
    if "nc" in _CACHE:
        return _CACHE["nc"]
    from concourse import bacc, mybir, tile
    dt = mybir.dt
    cn = _consts()
    nc = bacc.Bacc("TRN2", target_bir_lowering=False, debug=False,
                   enable_asserts=False, num_devices=N_CORES,
                   dynamic_dma_scratch_size=32768)
    dram = {}
    dram["feats"] = nc.dram_tensor("feats", [N_PER_CORE, ZROW + 2, C],
                                   dt.float16, kind="ExternalInput").ap()
    dram["gidx"] = nc.dram_tensor("gidx", [128, GIDX_COLS],
                                  dt.int16, kind="ExternalInput").ap()
    dram["offidx"] = nc.dram_tensor("offidx", [128, NBLK * 16], dt.int16,
                                    kind="ExternalInput").ap()
    dram["endsidx"] = nc.dram_tensor("endsidx", [128, NBLK * 12], dt.int16,
                                     kind="ExternalInput").ap()
    dram["ctab"] = nc.dram_tensor("ctab", [128, NBLK * 2 * RB * P], dt.float16,
                                  kind="ExternalInput").ap()
    dram["out"] = nc.dram_tensor("out", [N_PER_CORE, C, S, P], dt.float32,
                                 kind="ExternalOutput").ap()

    def dtf(a):
        if a.dtype == np.int16:
            return dt.int16
        n = str(a.dtype)
        if n == "bfloat16":
            return dt.bfloat16
        if n == "float16":
            return dt.float16
        return dt.float32

    for k, v in cn.items():
        dram[k] = nc.dram_tensor(f"c_{k}", list(v.shape), dtf(v),
                                 kind="ExternalInput").ap()

    with tile.TileContext(nc) as tc:
        with ExitStack() as stk:
            build_kernel_body(stk, tc, nc, dram)
    nc.compile()
    _CACHE["nc"] = nc
    _CACHE["consts"] = cn
    return nc


def _host_fallback(feats, part_labels, valid_mask, parts_num):
    n, c, s, k = feats.shape
    Pn = int(parts_num)
    f = np.asarray(feats, np.float32).transpose(0, 2, 3, 1).reshape(-1, c)
    seg = (np.asarray(part_labels).astype(np.int64).reshape(n * s, k)
           + np.arange(n * s, dtype=np.int64)[:, None] * Pn).reshape(-1)
    vm = np.asarray(valid_mask).reshape(-1).astype(np.float32)
    nsg = n * s * Pn
    psum = np.zeros((nsg, c), np.float32)
    np.add.at(psum, seg, f * vm[:, None])
    pcnt = np.zeros(nsg, np.float32)
    np.add.at(pcnt, seg, vm)
    patch = np.zeros(nsg, np.float32)
    np.add.at(patch, seg, np.ones_like(vm))
    smax = np.full((nsg, c), -np.inf, np.float32)
    np.maximum.at(smax, seg, f)
    pmax = np.where(patch[:, None] > 0, np.maximum(smax, -100.0), 0.0)
    pooled = psum / np.maximum(pcnt, 1.0)[:, None] + pmax
    return pooled.reshape(n, s, Pn, c).transpose(0, 3, 1, 2).astype(np.float32)


def kernel(feats, part_labels, valid_mask, parts_num):
    feats = np.ascontiguousarray(np.asarray(feats), dtype=np.float32)
    if int(parts_num) != P or feats.shape != (N, C, S, K) \
            or not bool(np.all(np.asarray(valid_mask))):
        return _host_fallback(feats, part_labels, valid_mask, parts_num)

    from concourse import bass_utils
    nc = build_nc()
    cn = _CACHE["consts"]

    lab = np.asarray(part_labels).astype(np.int64)
    if int(lab.min()) < 0 or int(lab.max()) >= P:
        return _host_fallback(feats, part_labels, valid_mask, parts_num)
    T = _host_tables(lab)
    # the Sdiff neighbor-shift needs a valid prefix-sum baseline in every
    # row's part-0 grid slot; an empty part 0 would corrupt part 1's mean
    if int(T["cntp"].min()) == 0:
        return _host_fallback(feats, part_labels, valid_mask, parts_num)
    # feats_t: [N, S*K+2, C] fp16, biased, zero rows appended
    ft = feats.transpose(0, 2, 3, 1).reshape(N, S * K, C) + BIAS
    ft = np.concatenate([ft, np.zeros((N, 2, C), np.float32)], 1)
    ft = ft.astype(np.float16)

    in_maps = []
    for core in range(N_CORES):
        ct = _core_tables(T, core)
        sl = slice(core * N_PER_CORE, (core + 1) * N_PER_CORE)
        m = {"feats": np.ascontiguousarray(ft[sl]),
             "gidx": ct["gidx"], "offidx": ct["offidx"],
             "endsidx": ct["endsidx"], "ctab": ct["ctab"]}
        for k, v in cn.items():
            m[f"c_{k}"] = v
        in_maps.append(m)

    res = bass_utils.run_bass_kernel_spmd(nc, in_maps, core_ids=list(range(N_CORES)))
    out = np.empty((N, C, S, P), np.float32)
    for core in range(N_CORES):
        out[core * N_PER_CORE:(core + 1) * N_PER_CORE] = res.results[core]["out"]
    return out


# revision 33
# speedup vs baseline: 1.8710x; 1.0298x over previous
"""Trainium2 Bass kernel: segment mean+max pooling (AnchorHeightPart).

Algorithm (per core, data-parallel over n: 4 n-batches/core):
  Host pre-sorts nothing value-wise; it builds, from the labels only, a
  counting-sort index table per (n,s) row with per-part counts padded up to
  multiples of 4 (hard bound 512 + 16*3 = 560 slots/row), pads pointing at an
  appended zero row. Values are biased +8 and cast fp16 so all real values are
  positive and zero-pads are neutral for both max and sum.

  Device: one transpose-mode dma_gather per n delivers the values sorted,
  padded, in [c, slot] layout straight from DRAM (DMA does load+permute in a
  single pass). Per row: PE broadcasts a 0/1 segment-reset mask (built by one
  gpsimd local_scatter per 6-row block), DVE runs a masked max-scan and a
  plain cumsum, Act downsamples both at the 4-grid (segment ends land on the
  grid by construction). Per block: one gpsimd ap_gather pulls both streams'
  segment-end values; five small DVE ops combine mean+max into the output.
"""

import os
import sys
from contextlib import ExitStack

import numpy as np

_REPO = "/opt/trn_rl_repo"
if _REPO not in sys.path and os.path.isdir(_REPO):
    sys.path.insert(0, _REPO)

N, C, S, K = 32, 128, 30, 512
P = 16
N_CORES = 8
N_PER_CORE = N // N_CORES          # 4
WPAD = 560                         # padded row width (hard max 512+16*3)
GRID = WPAD // 4                   # 140 grid slots per row
RB = 6                             # rows per block (6 of 8 lane groups used)
BPN = S // RB                      # 5 blocks per n
NBLK = N_PER_CORE * BPN            # 20 blocks per core
ZROW = S * K                       # zero-row index within an n's feats_t
NI = S * WPAD                      # 16800 gather idxs per n
NIP = 16896                        # padded to multiple of 128
BIAS = 8.0
GC = 768                           # idxs per dma_gather (HW-verified; 1024 crashes)
NCH = NIP // GC                    # 33 chunks per n
GIDX_COLS = 4224
SORTW = NIP

_CACHE = {}


def _consts():
    import ml_dtypes
    bf16 = ml_dtypes.bfloat16
    fp16 = np.float16
    q = np.arange(128)
    c = {}
    for r in range(RB):
        c[f"EErb{r}"] = (q[:, None] == 16 * r + q[None, :] % 16).astype(bf16)
    pm = np.ones((128, RB * P), fp16)
    pm[:, 0] = 0.0
    pm[:, 3 * P] = 0.0
    c["PM"] = pm
    c["ONE16"] = np.ones((128, 16), bf16)
    c["ZD0"] = np.zeros((128, 3 * WPAD), np.float32)
    return c


def _host_tables(lab):
    """lab: [N, S, K] int64 labels. Returns global table arrays."""
    oh = lab[..., None] == np.arange(P)
    counts = oh.sum(2)                            # [N,S,P]
    cntp = ((counts + 3) // 4) * 4
    offp = np.cumsum(cntp, axis=2) - cntp
    endp = offp + cntp - 1                        # == offp-1 when cntp == 0
    assert (offp[..., -1] + cntp[..., -1]).max() <= WPAD

    order = np.argsort(lab, axis=2, kind="stable")
    sortedlab = np.take_along_axis(lab, order, 2)
    cumx = np.cumsum(counts, axis=2) - counts
    rank = np.arange(K)[None, None, :] - np.take_along_axis(cumx, sortedlab, 2)
    slot = np.take_along_axis(offp, sortedlab, 2) + rank
    idxg = np.full((N, S, WPAD), ZROW, np.int64)
    np.put_along_axis(idxg, slot, order, axis=2)
    real = idxg != ZROW
    idxg = np.where(real, idxg + np.arange(S)[None, :, None] * K, ZROW)

    indic = (counts > 0).astype(np.float16)
    recip = np.where(counts > 0, 1.0 / np.maximum(counts, 1), 0.0).astype(np.float16)
    return dict(cntp=cntp, offp=offp, endp=endp, idxg=idxg,
                indic=indic, recip=recip)


def _core_tables(T, core):
    """Per-core DMA-ready tables."""
    n0 = core * N_PER_CORE
    # gather idx, wrapped i16, packed [128, 4*NIP//16].
    # n0 is split into a 1-block head (rows 0-5, 3456 idxs) + tail (rows
    # 6-29, 13440 idxs) so compute can start before the full gather lands.
    def wrap(flat, pad_to):
        flat = np.concatenate([flat, np.full(pad_to - len(flat), ZROW, np.int64)])
        w = flat.reshape(pad_to // 16, 16).T.astype(np.int16)
        return np.tile(w, (8, 1))                              # [128, pad/16]

    parts = [wrap(T["idxg"][n0 + ni].reshape(-1), NIP)
             for ni in range(N_PER_CORE)]
    gidx = np.concatenate(parts, axis=1)
    assert gidx.shape[1] == GIDX_COLS
    # per-block tables
    offidx = np.full((NBLK, 128, 16), -1, np.int16)
    endsidx = np.empty((NBLK, 128, 12), np.int16)
    ctab = np.empty((NBLK, 128, 2 * RB * P), np.float16)
    for b in range(NBLK):
        ni, bi = b // BPN, b % BPN
        n = n0 + ni
        for r in range(RB):
            s = bi * RB + r
            op_ = T["offp"][n, s]
            cp_ = T["cntp"][n, s]
            row = np.where(cp_ > 0, op_, -1).astype(np.int16)
            offidx[b, 16 * r:16 * (r + 1), :] = row[None, :]
        # ends idx: j = m*96 + r*16 + p -> r*280 + m*140 + max(endp//4, 0)
        vals = np.empty(2 * RB * P, np.int64)
        for m in range(2):
            for r in range(RB):
                s = bi * RB + r
                g = np.maximum(T["endp"][n, s] // 4, 0)
                vals[m * RB * P + r * P:(m * RB * P + r * P) + P] = \
                    (r // 3) * 6 * GRID + m * 3 * GRID + (r % 3) * GRID + g
        w = vals.reshape(12, 16).T.astype(np.int16)            # [16, 12]
        endsidx[b] = np.tile(w, (8, 1))
        ct = np.empty(2 * RB * P, np.float16)
        for r in range(RB):
            s = bi * RB + r
            ct[r * P:(r + 1) * P] = T["indic"][n, s]
            ct[RB * P + r * P:RB * P + (r + 1) * P] = T["recip"][n, s]
        ctab[b] = np.broadcast_to(ct, (128, 2 * RB * P))
    # pre-transpose to [128, NBLK*X] so the preload DMAs are contiguous
    offidx_t = np.ascontiguousarray(offidx.transpose(1, 0, 2).reshape(128, -1))
    endsidx_t = np.ascontiguousarray(endsidx.transpose(1, 0, 2).reshape(128, -1))
    ctab_t = np.ascontiguousarray(ctab.transpose(1, 0, 2).reshape(128, -1))
    return dict(gidx=gidx, offidx=offidx_t, endsidx=endsidx_t, ctab=ctab_t)


def build_kernel_body(stk, tc, nc, dram):
    from concourse import mybir
    from concourse.tile_rust import add_dep_helper
    dt = mybir.dt
    Alu = mybir.AluOpType
    Act = mybir.ActivationFunctionType
    f32, i16, h16, bf = dt.float32, dt.int16, dt.float16, dt.bfloat16

    feats_d = dram["feats"]       # [4, ZROW+2, C] fp16 (biased, zero rows)
    gidx_d = dram["gidx"]         # [4, 128, NIP//16] i16
    offidx_d = dram["offidx"]     # [NBLK, 128, 16] i16
    endsidx_d = dram["endsidx"]   # [NBLK, 128, 12] i16
    ctab_d = dram["ctab"]         # [NBLK, 128, 192] fp16
    out_d = dram["out"]           # [4, C, S, P] f32

    cpool = stk.enter_context(tc.tile_pool(name="consts", bufs=1))
    spool = stk.enter_context(tc.tile_pool(name="sorted", bufs=2))
    ipool = stk.enter_context(tc.tile_pool(name="idx", bufs=2))
    bpool = stk.enter_context(tc.tile_pool(name="bb", bufs=4))
    tpool = stk.enter_context(tc.tile_pool(name="tabs", bufs=3))
    ppool = stk.enter_context(tc.tile_pool(name="brow", bufs=2, space="PSUM"))
    mpool = stk.enter_context(tc.tile_pool(name="scan", bufs=3))
    dpool = stk.enter_context(tc.tile_pool(name="down", bufs=3))
    gpool = stk.enter_context(tc.tile_pool(name="ends", bufs=3))
    opool = stk.enter_context(tc.tile_pool(name="oacc", bufs=2))

    def ldconst(name, dtype=f32):
        a = dram[name]
        t = cpool.tile(list(a.shape), dtype, tag=name)
        nc.sync.dma_start(out=t[:], in_=a[:])
        return t

    # preload everything up front, most-urgent first, so no DMA queues
    # behind the big feats gathers during steady state
    gidx_all = cpool.tile([128, GIDX_COLS], i16, tag="gidx_all")
    nc.sync.dma_start(out=gidx_all[:, 0:432], in_=gidx_d[:, 0:432])
    nc.sync.dma_start(out=gidx_all[:, 432:GIDX_COLS], in_=gidx_d[:, 432:GIDX_COLS])
    ONE16 = ldconst("ONE16", dtype=bf)
    off_all = cpool.tile([128, NBLK * 16], i16, tag="off_all")
    nc.sync.dma_start(out=off_all[:], in_=offidx_d[:])
    EErb = [ldconst(f"EErb{r}", dtype=bf) for r in range(RB)]
    PM = ldconst("PM", dtype=h16)
    ZD0 = ldconst("ZD0", dtype=f32)
    ends_all = cpool.tile([128, NBLK * 12], i16, tag="ends_all")
    nc.sync.dma_start(out=ends_all[:], in_=endsidx_d[:])
    ctab_all = cpool.tile([128, NBLK * 2 * RB * P], h16, tag="ctab_all")
    nc.sync.dma_start(out=ctab_all[:], in_=ctab_d[:])

    last_pool_op = [None]

    def chain_pool(inst):
        if last_pool_op[0] is not None:
            add_dep_helper(inst.ins, last_pool_op[0].ins, False,
                           "pool library phase order")
        last_pool_op[0] = inst

    sortv_t = {}
    oacc_t = {}
    bbinv_t = {}
    ends_t = {}

    next_chunk = {}

    def issue_chunks(ni, upto):
        """Issue 512-idx gather chunks for n=ni until `upto` chunks done."""
        if ni not in sortv_t:
            sortv_new = spool.tile([128, SORTW], h16, tag="sortv")
            sortv_t[ni] = sortv_new
            next_chunk[ni] = 0
        sortv = sortv_t[ni]
        upto = min(upto, NCH)
        for ch in range(next_chunk[ni], upto):
            o0 = ch * GC
            g_i = nc.gpsimd.dma_gather(
                out_ap=sortv[:, o0:o0 + GC].rearrange("c (o n) -> c o n", o=1),
                in_ap=feats_d[ni],
                idxs_ap=gidx_all[:, ni * (NIP // 16) + ch * (GC // 16):
                                 ni * (NIP // 16) + (ch + 1) * (GC // 16)],
                num_idxs=GC, num_idxs_reg=GC,
                elem_size=C, transpose=True)
            chain_pool(g_i)
        next_chunk[ni] = max(next_chunk[ni], upto)

    def issue_mask(b):
        bb = bpool.tile([128, WPAD], bf, tag="bbt")
        sc_i = nc.gpsimd.local_scatter(
            out_ap=bb[:], data_ap=ONE16[:], idxs_ap=off_all[:, b * 16:(b + 1) * 16],
            channels=128, num_elems=WPAD, num_idxs=16)
        chain_pool(sc_i)
        bbinv = bpool.tile([128, WPAD], bf, tag="bbinvt")
        nc.scalar.activation(out=bbinv[:], in_=bb[:], func=Act.Identity,
                             bias=1.0, scale=-1.0)
        bbinv_t[b] = bbinv

    def issue_rows_and_ends(b):
        ni, bi = b // BPN, b % BPN
        sortv = sortv_t[ni]
        bbinv = bbinv_t.pop(b)
        endt = ends_all[:, b * 12:(b + 1) * 12]
        D = dpool.tile([128, RB * 2 * GRID], f32, tag="D")
        W3 = 3 * WPAD
        for g3 in range(2):
            # [128, 2048] f32 = exactly 4 PSUM banks, so 2 bufs fill PSUM and
            # every tile is bank-aligned; matmul writes must not cross the
            # 512-col bank lines, so split each row's mask at them.
            brow = ppool.tile([128, 2048], f32, tag="brow")
            for rr in range(3):
                r = g3 * 3 + rr
                a, b_ = rr * WPAD, (rr + 1) * WPAD
                cut = ((a // 512) + 1) * 512
                nc.tensor.matmul(brow[:, a:cut], lhsT=EErb[r][:],
                                 rhs=bbinv[:, 0:cut - a],
                                 start=True, stop=True)
                nc.tensor.matmul(brow[:, cut:b_], lhsT=EErb[r][:],
                                 rhs=bbinv[:, cut - a:WPAD],
                                 start=True, stop=True)
            s = bi * RB + g3 * 3
            xo = s * WPAD
            xsl = sortv[:, xo:xo + W3]
            maxo = mpool.tile([128, W3], h16, tag="maxo")
            nc.vector.tensor_tensor_scan(
                out=maxo[:], data0=brow[:, 0:W3], data1=xsl, initial=0.0,
                op0=Alu.mult, op1=Alu.max)
            sumo = mpool.tile([128, W3], f32, tag="sumo")
            nc.vector.tensor_tensor_scan(
                out=sumo[:], data0=ZD0[:], data1=xsl, initial=0.0,
                op0=Alu.add, op1=Alu.add)
            d0 = g3 * 6 * GRID
            mview = maxo[:].rearrange("c (g f) -> c g f", f=4)[:, :, 3]
            nc.scalar.copy(out=D[:, d0:d0 + 3 * GRID], in_=mview)
            sview = sumo[:].rearrange("c (g f) -> c g f", f=4)[:, :, 3]
            nc.scalar.copy(out=D[:, d0 + 3 * GRID:d0 + 6 * GRID], in_=sview)
        Gt = gpool.tile([128, 2 * RB * P], f32, tag="Gt")
        g2_i = nc.gpsimd.ap_gather(
            out_ap=Gt[:], in_ap=D[:], idxs_ap=endt,
            channels=128, num_elems=RB * 2 * GRID, d=1, num_idxs=2 * RB * P)
        chain_pool(g2_i)
        ends_t[b] = Gt

    def issue_combine(b):
        ni, bi = b // BPN, b % BPN
        Gt = ends_t.pop(b)
        ctt = ctab_all[:, b * 2 * RB * P:(b + 1) * 2 * RB * P]
        if bi == 0:
            oacc_new = opool.tile([128, S * P], f32, tag="oacc")
            oacc_t[ni] = oacc_new
        oacc = oacc_t[ni]
        NP = RB * P  # 96
        u = gpool.tile([128, NP], f32, tag="u")
        nc.vector.scalar_tensor_tensor(
            out=u[:], in0=Gt[:, 0:NP], scalar=-2.0 * BIAS,
            in1=ctt[:, 0:NP], op0=Alu.add, op1=Alu.mult)
        v = gpool.tile([128, NP], f32, tag="v")
        nc.vector.tensor_tensor(out=v[:], in0=Gt[:, NP - 1:2 * NP - 1],
                                in1=PM[:], op=Alu.mult)
        w = gpool.tile([128, NP], f32, tag="w")
        nc.vector.tensor_tensor(out=w[:], in0=Gt[:, NP:2 * NP],
                                in1=v[:], op=Alu.subtract)
        x2 = gpool.tile([128, NP], f32, tag="x2")
        nc.vector.tensor_tensor(out=x2[:], in0=w[:],
                                in1=ctt[:, NP:2 * NP], op=Alu.mult)
        nc.vector.tensor_tensor(out=oacc[:, bi * NP:(bi + 1) * NP],
                                in0=u[:], in1=x2[:], op=Alu.add)
        if bi == BPN - 1:
            nc.sync.dma_start(out=out_d[ni].rearrange("c s p -> c (s p)"),
                              in_=oacc_t.pop(ni)[:])

    # software-pipelined schedule: masks two blocks ahead; gather chunks
    # issued with one-block lookahead, next n's chunks trickled in early
    def cover(bi):
        return -(-((bi + 1) * RB * WPAD) // GC)   # chunks covering block bi

    issue_chunks(0, cover(0))
    issue_mask(0)
    issue_mask(1)
    for b in range(NBLK):
        ni, bi = b // BPN, b % BPN
        if b + 2 < NBLK:
            issue_mask(b + 2)
        issue_chunks(ni, cover(bi + 1))
        if bi >= 1 and ni + 1 < N_PER_CORE:
            issue_chunks(ni + 1, bi * 6)
        issue_rows_and_ends(b)
        if b >= 1:
            issue_combine(b - 1)
    issue_combine(NBLK - 1)


def build_nc():
    if "nc" in _CACHE:
        return _CACHE["nc"]
    from concourse import bacc, mybir, tile
    dt = mybir.dt
    cn = _consts()
    nc = bacc.Bacc("TRN2", target_bir_lowering=False, debug=False,
                   enable_asserts=False, num_devices=N_CORES,
                   dynamic_dma_scratch_size=32768)
    dram = {}
    dram["feats"] = nc.dram_tensor("feats", [N_PER_CORE, ZROW + 2, C],
                                   dt.float16, kind="ExternalInput").ap()
    dram["gidx"] = nc.dram_tensor("gidx", [128, GIDX_COLS],
                                  dt.int16, kind="ExternalInput").ap()
    dram["offidx"] = nc.dram_tensor("offidx", [128, NBLK * 16], dt.int16,
                                    kind="ExternalInput").ap()
    dram["endsidx"] = nc.dram_tensor("endsidx", [128, NBLK * 12], dt.int16,
                                     kind="ExternalInput").ap()
    dram["ctab"] = nc.dram_tensor("ctab", [128, NBLK * 2 * RB * P], dt.float16,
                                  kind="ExternalInput").ap()
    dram["out"] = nc.dram_tensor("out", [N_PER_CORE, C, S, P], dt.float32,
                                 kind="ExternalOutput").ap()

    def dtf(a):
        if a.dtype == np.int16:
            return dt.int16
        n = str(a.dtype)
        if n == "bfloat16":
            return dt.bfloat16
        if n == "float16":
            return dt.float16
        return dt.float32

    for k, v in cn.items():
        dram[k] = nc.dram_tensor(f"c_{k}", list(v.shape), dtf(v),
                                 kind="ExternalInput").ap()

    with tile.TileContext(nc) as tc:
        with ExitStack() as stk:
            build_kernel_body(stk, tc, nc, dram)
    nc.compile()
    _CACHE["nc"] = nc
    _CACHE["consts"] = cn
    return nc


def _host_fallback(feats, part_labels, valid_mask, parts_num):
    n, c, s, k = feats.shape
    Pn = int(parts_num)
    f = np.asarray(feats, np.float32).transpose(0, 2, 3, 1).reshape(-1, c)
    seg = (np.asarray(part_labels).astype(np.int64).reshape(n * s, k)
           + np.arange(n * s, dtype=np.int64)[:, None] * Pn).reshape(-1)
    vm = np.asarray(valid_mask).reshape(-1).astype(np.float32)
    nsg = n * s * Pn
    psum = np.zeros((nsg, c), np.float32)
    np.add.at(psum, seg, f * vm[:, None])
    pcnt = np.zeros(nsg, np.float32)
    np.add.at(pcnt, seg, vm)
    patch = np.zeros(nsg, np.float32)
    np.add.at(patch, seg, np.ones_like(vm))
    smax = np.full((nsg, c), -np.inf, np.float32)
    np.maximum.at(smax, seg, f)
    pmax = np.where(patch[:, None] > 0, np.maximum(smax, -100.0), 0.0)
    pooled = psum / np.maximum(pcnt, 1.0)[:, None] + pmax
    return pooled.reshape(n, s, Pn, c).transpose(0, 3, 1, 2).astype(np.float32)


def kernel(feats, part_labels, valid_mask, parts_num):
    feats = np.ascontiguousarray(np.asarray(feats), dtype=np.float32)
    if int(parts_num) != P or feats.shape != (N, C, S, K) \
            or not bool(np.all(np.asarray(valid_mask))):
        return _host_fallback(feats, part_labels, valid_mask, parts_num)

    from concourse import bass_utils
    nc = build_nc()
    cn = _CACHE["consts"]

    lab = np.asarray(part_labels).astype(np.int64)
    if int(lab.min()) < 0 or int(lab.max()) >= P:
        return _host_fallback(feats, part_labels, valid_mask, parts_num)
    T = _host_tables(lab)
    # the Sdiff neighbor-shift needs a valid prefix-sum baseline in every
    # row's part-0 grid slot; an empty part 0 would corrupt part 1's mean
    if int(T["cntp"].min()) == 0:
        return _host_fallback(feats, part_labels, valid_mask, parts_num)
    # feats_t: [N, S*K+2, C] fp16, biased, zero rows appended
    ft = feats.transpose(0, 2, 3, 1).reshape(N, S * K, C) + BIAS
    ft = np.concatenate([ft, np.zeros((N, 2, C), np.float32)], 1)
    ft = ft.astype(np.float16)

    in_maps = []
    for core in range(N_CORES):
        ct = _core_tables(T, core)
        sl = slice(core * N_PER_CORE, (core + 1) * N_PER_CORE)
        m = {"feats": np.ascontiguousarray(ft[sl]),
             "gidx": ct["gidx"], "offidx": ct["offidx"],
             "endsidx": ct["endsidx"], "ctab": ct["ctab"]}
        for k, v in cn.items():
            m[f"c_{k}"] = v
        in_maps.append(m)

    res = bass_utils.run_bass_kernel_spmd(nc, in_maps, core_ids=list(range(N_CORES)))
    out = np.empty((N, C, S, P), np.float32)
    for core in range(N_CORES):
        out[core * N_PER_CORE:(core + 1) * N_PER_CORE] = res.results[core]["out"]
    return out
